# revision 3
# baseline (speedup 1.0000x reference)
"""Trainium2 Bass kernel for nn_DA_conv1D (dynamic depthwise conv1d + 1x1 conv
+ channel-attention gate), data-parallel over batch on 8 NeuronCores.

Shapes (hardcoded): x0 [32, 64, 16384] f32, x1 [32, 64] f32.
Each core handles 4 samples, organized as 2 "pairs" of 2 samples so the
128 SBUF partitions hold (2 samples x 64 channels).

v3 design:
  * output stored in bf16, widened to f32 on host (halves store traffic).
  * DMA bodies are exactly 4096B per partition row (2048 cols bf16) and
    aligned — the 4100B packets of the +halo scheme ran ~20% slower.
    The 2 halo columns arrive via separate tiny DMAs on the gpsimd queue.
  * the two sample-pairs are interleaved chunk-by-chunk: two independent
    streams fill each other's pipeline bubbles (fill/drain and PSUM-dep
    stalls).
  * per-chunk engine balancing:
      tap mode   P: all 3 depthwise taps as PE diag-matmuls
                 V: taps 1,2 on PE; tap 0 folded into a DVE STT that
                    also evacuates ps1 to SBUF (Prelu then reads SBUF)
      evac mode  D: DVE STT reads conv PSUM directly (1x mode)
                 E: ACT copies conv PSUM to SBUF bf16, DVE STT then runs
                    all-SBUF at 2x
    Patterns chosen so PE / ACT / DVE land at roughly equal busy time.

Per 1024-wide subtile (SBUF layout [128 part, L free]):
  ps1 = sum_j diag(kern_j) @ x_shift_j   (PE, accumulating bf16 matmuls)
  [V]  dwf = k0*x_{-1} + ps1             (DVE STT, PSUM->SBUF bf16)
  lr  = Prelu(ps1|dwf + pb)              (ACT, bf16 out)
  ps2 = blockdiag(conv_w) @ lr           (PE bf16 matmuls)
  [E]  es  = copy(ps2)                   (ACT, PSUM->SBUF bf16)
  out = x0b * att + (ps2|es)             (DVE STT -> bf16 SBUF)

x0 is pre-shifted by d = conv_b/att and cast to bf16 on host; the gate
multiply then yields att*x0 + conv_b for free, and the Prelu bias
pb = -sum_j kern_j*d compensates the shift on the depthwise path.
"""

import os
import sys

for _p in ("/opt/trn_rl_repo", "/root/.axon_site/_ro/trn_rl_repo"):
    if os.path.isdir(_p) and _p not in sys.path:
        sys.path.append(_p)

import ml_dtypes
import numpy as np

import concourse.bacc as bacc
import concourse.tile as tile
from concourse import mybir
from concourse.bass_utils import run_bass_kernel_spmd

B, C, L, K = 32, 64, 16384, 3
N_CORES = 8
SAMPLES_PER_CORE = B // N_CORES          # 4
PAIRS = SAMPLES_PER_CORE // 2            # 2
P = 128                                  # SBUF partitions = 2 samples x 64 ch
CHUNK = 2048                             # max chunk (SBUF tile size)
# 2048-col bodies keep every DMA packet at exactly 4096B; 1024 at the edges
# shrink pipeline fill/drain
CHUNK_SIZES = [1024] + [2048] * 7 + [1024]
ST = 1024                                # ACT/DVE subtile width (2 PSUM banks)
NT = 512                                 # matmul moving width (PSUM bank)
F32 = mybir.dt.float32
BF16 = mybir.dt.bfloat16
BF16_NP = ml_dtypes.bfloat16

N_CHUNKS = len(CHUNK_SIZES) * PAIRS      # 18 interleaved chunks
# per-chunk tap0 placement: P = tap0 on the PE, V = folded into DVE STT
TAP_PATTERN = "PVVPVVPVVPVVPVVPVV"
# per-chunk conv-PSUM evacuation: D = DVE STT reads PSUM, E = ACT copies
# PSUM->SBUF first
EVAC_PATTERN = "DEDEDEDEDEDEDEDEDE"

TRACE = False          # test harness flips this to profile
USE_LRELU = True       # HW Prelu activation (CoreSim lacks it; see simcheck)
LAST_RESULT = None     # BassKernelResults of the most recent run

_COMPILED = {}


def _subtiles(csz):
    if csz <= ST:
        return [(0, csz)]
    return [(0, ST), (ST, csz - ST)]


def _build_program(use_lrelu, tap_pattern, evac_pattern):
    nc = bacc.Bacc("TRN2", target_bir_lowering=False, debug=False,
                   num_devices=N_CORES)

    x0b = nc.dram_tensor("x0b", [PAIRS, P, L], BF16,
                         kind="ExternalInput").ap()
    # diag kernels pre-flattened per partition: [(pair, tap) -> 128 cols]
    diags = nc.dram_tensor("diags", [P, PAIRS * K * P], BF16,
                           kind="ExternalInput").ap()
    # scal columns: [att(pair), pb(pair), k0(pair)] where pb is the prelu
    # bias compensating the host-side x0 + d shift, k0 the tap-0 kernel
    scal = nc.dram_tensor("scal", [P, 3 * PAIRS], F32,
                          kind="ExternalInput").ap()
    # bf16(d) per pair: halo fill value so padded taps cancel exactly
    dcol = nc.dram_tensor("dcol", [PAIRS, P, 1], BF16,
                          kind="ExternalInput").ap()
    wblk = nc.dram_tensor("wblk", [P, P], BF16, kind="ExternalInput").ap()
    out = nc.dram_tensor("out", [PAIRS, P, L], BF16, kind="ExternalOutput").ap()

    mult = mybir.AluOpType.mult
    add = mybir.AluOpType.add
    Relu = mybir.ActivationFunctionType.Relu
    Prelu = mybir.ActivationFunctionType.Prelu
    Ident = mybir.ActivationFunctionType.Identity

    # interleave the two pairs: (pair, lo, csz, tap_mode, evac_mode)
    chunks = []
    for c, csz in enumerate(CHUNK_SIZES):
        lo = sum(CHUNK_SIZES[:c])
        for p in range(PAIRS):
            chunks.append((p, lo, csz))
    chunks = [c + (tap_pattern[i], evac_pattern[i])
              for i, c in enumerate(chunks)]

    with tile.TileContext(nc) as tc:
        with (
            tc.tile_pool(name="consts", bufs=1) as consts,
            tc.tile_pool(name="xbf", bufs=6) as xbf_pool,
            tc.tile_pool(name="dwf", bufs=3) as dwf_pool,
            tc.tile_pool(name="lr", bufs=5) as lr_pool,
            tc.tile_pool(name="es", bufs=3) as es_pool,
            tc.tile_pool(name="r9", bufs=4) as r9_pool,
            tc.tile_pool(name="outc", bufs=4) as out_pool,
            tc.tile_pool(name="ps1", bufs=2, space="PSUM") as ps1_pool,
            tc.tile_pool(name="ps2", bufs=2, space="PSUM") as ps2_pool,
        ):
            # first two chunk loads issued before the const DMAs so the
            # input stream starts immediately
            pre = []
            for i in range(2):
                p, lo, csz, _, _ = chunks[i]
                xbf = xbf_pool.tile([P, CHUNK + 4], BF16, tag="xbf")
                nc.sync.dma_start(xbf[:, 2:csz + 2], x0b[p, :, 0:csz])
                nc.gpsimd.dma_start(xbf[:, 1:2], dcol[p])
                nc.gpsimd.dma_start(xbf[:, csz + 2:csz + 3],
                                    x0b[p, :, csz:csz + 1])
                pre.append(xbf)

            diag_t = consts.tile([P, PAIRS * K * P], BF16)
            nc.scalar.dma_start(diag_t[:], diags[:])
            wblk_t = consts.tile([P, P], BF16)
            nc.scalar.dma_start(wblk_t[:], wblk[:])
            scal_t = consts.tile([P, 3 * PAIRS], F32)
            nc.scalar.dma_start(scal_t[:], scal[:])
            att = [scal_t[:, p:p + 1] for p in range(PAIRS)]
            pb = [scal_t[:, PAIRS + p:PAIRS + p + 1] for p in range(PAIRS)]
            k0 = [scal_t[:, 2 * PAIRS + p:2 * PAIRS + p + 1]
                  for p in range(PAIRS)]

            def lrelu(dst, src, bias):
                if use_lrelu:
                    nc.scalar.activation(dst, src, Prelu, bias=bias,
                                         alpha=0.1)
                else:
                    sz = dst.shape[-1]
                    tt = r9_pool.tile([P, ST], F32, tag="tt")
                    nc.scalar.activation(tt[:, :sz], src, Ident, bias=bias)
                    r9 = r9_pool.tile([P, ST], F32, tag="r9")
                    nc.scalar.activation(r9[:, :sz], tt[:, :sz], Relu,
                                         scale=0.9)
                    nc.vector.scalar_tensor_tensor(
                        dst, tt[:, :sz], 0.1, r9[:, :sz], op0=mult, op1=add)

            def stage1(c):
                p, lo, csz, tmode, _ = chunks[c]
                if c < 2:
                    xbf = pre[c]
                else:
                    xbf = xbf_pool.tile([P, CHUNK + 4], BF16, tag="xbf")
                    # aligned body, halo columns separately
                    nc.sync.dma_start(xbf[:, 2:csz + 2],
                                      x0b[p, :, lo:lo + csz])
                    if lo == 0:
                        nc.gpsimd.dma_start(xbf[:, 1:2], dcol[p])
                    else:
                        nc.gpsimd.dma_start(xbf[:, 1:2],
                                            x0b[p, :, lo - 1:lo])
                    if lo + csz == L:
                        nc.gpsimd.dma_start(xbf[:, csz + 2:csz + 3],
                                            dcol[p])
                    else:
                        nc.gpsimd.dma_start(
                            xbf[:, csz + 2:csz + 3],
                            x0b[p, :, lo + csz:lo + csz + 1])

                lrs = []
                for off, sz in _subtiles(csz):
                    ps1 = ps1_pool.tile([P, ST], F32)
                    jl = (0, 1, 2) if tmode == "P" else (1, 2)
                    for j in jl:
                        for hlo in range(0, sz, NT):
                            hsz = min(NT, sz - hlo)
                            nc.tensor.matmul(
                                ps1[:, hlo:hlo + hsz],
                                diag_t[:, (p * K + j) * P:
                                       (p * K + j + 1) * P],
                                xbf[:, off + hlo + 1 + j:
                                    off + hlo + 1 + j + hsz],
                                start=(j == jl[0]), stop=(j == 2),
                            )
                    lr = lr_pool.tile([P, ST], BF16)
                    if tmode == "V":
                        dwf = dwf_pool.tile([P, ST], BF16)
                        nc.vector.scalar_tensor_tensor(
                            dwf[:, :sz], xbf[:, off + 1:off + 1 + sz],
                            k0[p], ps1[:, :sz], op0=mult, op1=add)
                        lrelu(lr[:, :sz], dwf[:, :sz], pb[p])
                    else:
                        lrelu(lr[:, :sz], ps1[:, :sz], pb[p])
                    lrs.append(lr)
                return xbf, lrs

            def stage2(c, xbf, lrs):
                p, lo, csz, _, emode = chunks[c]
                outc = out_pool.tile([P, CHUNK], BF16, tag="outc")
                for (off, sz), lr in zip(_subtiles(csz), lrs):
                    ps2 = ps2_pool.tile([P, ST], F32)
                    for hlo in range(0, sz, NT):
                        hsz = min(NT, sz - hlo)
                        nc.tensor.matmul(
                            ps2[:, hlo:hlo + hsz], wblk_t[:],
                            lr[:, hlo:hlo + hsz], start=True, stop=True)
                    if emode == "E":
                        es = es_pool.tile([P, ST], BF16)
                        nc.scalar.activation(es[:, :sz], ps2[:, :sz], Ident)
                        src1 = es[:, :sz]
                    else:
                        src1 = ps2[:, :sz]
                    nc.vector.scalar_tensor_tensor(
                        outc[:, off:off + sz],
                        xbf[:, off + 2:off + 2 + sz],
                        att[p], src1, op0=mult, op1=add)
                nc.gpsimd.dma_start(out[p, :, lo:lo + csz], outc[:, :csz])

            prev = None
            for c in range(len(chunks)):
                cur = stage1(c)
                if prev is not None:
                    stage2(c - 1, *prev)
                prev = cur
            stage2(len(chunks) - 1, *prev)

    nc.compile()
    return nc


def _lrelu(x):
    return np.where(x >= 0, x, np.float32(0.1) * x)


def kernel(x0, x1, W1, W2, conv_w, conv_b, ca_w1, ca_w2):
    global LAST_RESULT
    x0 = np.ascontiguousarray(np.asarray(x0, dtype=np.float32))
    x1 = np.asarray(x1, dtype=np.float32)
    W1 = np.asarray(W1, dtype=np.float32)
    W2 = np.asarray(W2, dtype=np.float32)
    conv_w = np.asarray(conv_w, dtype=np.float32)
    conv_b = np.asarray(conv_b, dtype=np.float32)
    ca_w1 = np.asarray(ca_w1, dtype=np.float32)
    ca_w2 = np.asarray(ca_w2, dtype=np.float32)

    # dynamic depthwise kernels + SE gate (tiny, fp32 host math)
    h = _lrelu(x1 @ W1.T)                                   # [B, 64]
    kern = (h @ W2.T).reshape(B, C, K)                      # [B, C, K]
    att = 1.0 / (1.0 + np.exp(-(_lrelu(x1 @ ca_w1.T) @ ca_w2.T)))
    att = att.astype(np.float32)                            # [B, C]

    # block-diagonal 1x1-conv weight as lhsT: lhsT[k, m] = W[m, k]
    wblk_np = np.zeros((P, P), np.float32)
    wblk_np[:C, :C] = conv_w.T
    wblk_np[C:, C:] = conv_w.T
    wblk_np = wblk_np.astype(BF16_NP)

    key = (USE_LRELU, TAP_PATTERN, EVAC_PATTERN)
    if key not in _COMPILED:
        _COMPILED[key] = _build_program(*key)
    nc = _COMPILED[key]

    biasP = np.tile(conv_b, 2).astype(np.float32)            # [P]
    in_maps = []
    for core in range(N_CORES):
        s0 = core * SAMPLES_PER_CORE
        diags_np = np.zeros((P, PAIRS * K * P), np.float32)
        scal_np = np.empty((P, 3 * PAIRS), np.float32)
        dcol_np = np.empty((PAIRS, P, 1), np.float32)
        dvals = np.empty((PAIRS, P), np.float32)
        for p in range(PAIRS):
            ka = kern[s0 + 2 * p]          # [C, K]
            kb = kern[s0 + 2 * p + 1]
            kern_bf = np.empty((P, K), np.float32)
            for j in range(K):
                s = (p * K + j) * P
                d = np.concatenate([ka[:, j], kb[:, j]])
                np.fill_diagonal(diags_np[:, s:s + P], d)
                kern_bf[:, j] = d.astype(BF16_NP).astype(np.float32)
            attp = np.concatenate([att[s0 + 2 * p], att[s0 + 2 * p + 1]])
            dp = biasP / attp                                 # [P]
            dvals[p] = dp
            dcol_np[p, :, 0] = dp
            scal_np[:, p] = attp
            # depthwise compensation: -sum_j bf16(kern_j) * d
            scal_np[:, PAIRS + p] = -(kern_bf.sum(axis=1) * dp)
            scal_np[:, 2 * PAIRS + p] = kern_bf[:, 0]
        x0c = x0[s0:s0 + SAMPLES_PER_CORE].reshape(PAIRS, P, L)
        x0c = (x0c + dvals[:, :, None]).astype(BF16_NP)
        in_maps.append({
            "x0b": x0c,
            "diags": diags_np.astype(BF16_NP),
            "scal": scal_np,
            "dcol": dcol_np.astype(BF16_NP),
            "wblk": wblk_np,
        })

    res = run_bass_kernel_spmd(nc, in_maps, list(range(N_CORES)), trace=TRACE)
    LAST_RESULT = res

    full = np.empty((B, C, L), np.float32)
    for core in range(N_CORES):
        s0 = core * SAMPLES_PER_CORE
        full[s0:s0 + SAMPLES_PER_CORE] = (
            res.results[core]["out"].astype(np.float32)
            .reshape(SAMPLES_PER_CORE, C, L))
    return full


# revision 12
# speedup vs baseline: 1.0900x; 1.0900x over previous
"""Trainium2 Bass kernel for nn_DA_conv1D (dynamic depthwise conv1d + 1x1 conv
+ channel-attention gate), data-parallel over batch on 8 NeuronCores.

Shapes (hardcoded): x0 [32, 64, 16384] f32, x1 [32, 64] f32.
Each core handles 4 samples, organized as 2 "pairs" of 2 samples so the
128 SBUF partitions hold (2 samples x 64 channels).

v3 design:
  * output stored in bf16, widened to f32 on host (halves store traffic).
  * DMA bodies are exactly 4096B per partition row (2048 cols bf16) and
    aligned — the 4100B packets of the +halo scheme ran ~20% slower.
    The 2 halo columns arrive via separate tiny DMAs on the gpsimd queue.
  * the two sample-pairs are interleaved chunk-by-chunk: two independent
    streams fill each other's pipeline bubbles (fill/drain and PSUM-dep
    stalls).
  * per-chunk engine balancing:
      tap mode   P: all 3 depthwise taps as PE diag-matmuls
                 V: taps 1,2 on PE; tap 0 folded into a DVE STT that
                    also evacuates ps1 to SBUF (Prelu then reads SBUF)
      evac mode  D: DVE STT reads conv PSUM directly (1x mode)
                 E: ACT copies conv PSUM to SBUF bf16, DVE STT then runs
                    all-SBUF at 2x
    Patterns chosen so PE / ACT / DVE land at roughly equal busy time.

Per 1024-wide subtile (SBUF layout [128 part, L free]):
  ps1 = sum_j diag(kern_j) @ x_shift_j   (PE, accumulating bf16 matmuls)
  [V]  dwf = k0*x_{-1} + ps1             (DVE STT, PSUM->SBUF bf16)
  lr  = Prelu(ps1|dwf + pb)              (ACT, bf16 out)
  ps2 = blockdiag(conv_w) @ lr           (PE bf16 matmuls)
  [E]  es  = copy(ps2)                   (ACT, PSUM->SBUF bf16)
  out = x0b * att + (ps2|es)             (DVE STT -> bf16 SBUF)

x0 is pre-shifted by d = conv_b/att and cast to bf16 on host; the gate
multiply then yields att*x0 + conv_b for free, and the Prelu bias
pb = -sum_j kern_j*d compensates the shift on the depthwise path.
"""

import os
import sys

for _p in ("/opt/trn_rl_repo", "/root/.axon_site/_ro/trn_rl_repo"):
    if os.path.isdir(_p) and _p not in sys.path:
        sys.path.append(_p)

import ml_dtypes
import numpy as np

import concourse.bacc as bacc
import concourse.tile as tile
from concourse import mybir
from concourse.bass_utils import run_bass_kernel_spmd

B, C, L, K = 32, 64, 16384, 3
N_CORES = 8
SAMPLES_PER_CORE = B // N_CORES          # 4
PAIRS = SAMPLES_PER_CORE // 2            # 2
P = 128                                  # SBUF partitions = 2 samples x 64 ch
CHUNK = 4096                             # max chunk (SBUF tile size)
# tapered; the input rows for each chunk are host-packed with their 2+2
# halo columns and padded to a 32-col (64B) multiple so every DMA packet
# is a 64B multiple (4100B packets measured ~20% slower than 4096B)
CHUNK_SIZES = [1024, 2048, 4096, 4096, 4096, 1024]
PACKED = {c: (c + 4 + 31) // 32 * 32 for c in CHUNK_SIZES}
PCOLS = max(PACKED.values())             # 4128 (8256B rows)
ST = 1024                                # ACT/DVE subtile width (2 PSUM banks)
NT = 512                                 # matmul moving width (PSUM bank)
F32 = mybir.dt.float32
BF16 = mybir.dt.bfloat16
BF16_NP = ml_dtypes.bfloat16

NCH = len(CHUNK_SIZES)                   # 6 chunks per pair
# per-chunk tap0 placement: P = tap0 on the PE, V = folded into DVE STT
# (global order interleaves the two pairs chunk-by-chunk)
TAP_PATTERN = "PVVPPVVPPVVP"

TRACE = False          # test harness flips this to profile
USE_LRELU = True       # HW Prelu activation (CoreSim lacks it; see simcheck)
LAST_RESULT = None     # BassKernelResults of the most recent run

_COMPILED = {}


def _subtiles(csz):
    return [(o, min(ST, csz - o)) for o in range(0, csz, ST)]


def _build_program(use_lrelu, tap_pattern):
    nc = bacc.Bacc("TRN2", target_bir_lowering=False, debug=False,
                   num_devices=N_CORES)

    # host-packed input: chunk c's rows hold x0d[lo-2 : lo+csz+2] (+pad),
    # so xbf[:, i] = x0d[lo + i - 2]
    x0p = nc.dram_tensor("x0p", [PAIRS, NCH, P, PCOLS], BF16,
                         kind="ExternalInput").ap()
    # diag kernels pre-flattened per partition: [(pair, tap) -> 128 cols]
    diags = nc.dram_tensor("diags", [P, PAIRS * K * P], BF16,
                           kind="ExternalInput").ap()
    # scal columns: [att(pair), pb(pair), k0(pair)] where pb is the prelu
    # bias compensating the host-side x0 + d shift, k0 the tap-0 kernel
    scal = nc.dram_tensor("scal", [P, 3 * PAIRS], F32,
                          kind="ExternalInput").ap()
    wblk = nc.dram_tensor("wblk", [P, P], BF16, kind="ExternalInput").ap()
    out = nc.dram_tensor("out", [PAIRS, P, L], BF16, kind="ExternalOutput").ap()

    mult = mybir.AluOpType.mult
    add = mybir.AluOpType.add
    Relu = mybir.ActivationFunctionType.Relu
    Prelu = mybir.ActivationFunctionType.Prelu
    Ident = mybir.ActivationFunctionType.Identity

    # interleave the two pairs: (pair, chunk_idx, lo, csz, tap_mode)
    chunks = []
    for c, csz in enumerate(CHUNK_SIZES):
        lo = sum(CHUNK_SIZES[:c])
        for p in range(PAIRS):
            chunks.append((p, c, lo, csz))
    chunks = [ch + (tap_pattern[i],) for i, ch in enumerate(chunks)]

    with tile.TileContext(nc) as tc:
        with (
            tc.tile_pool(name="consts", bufs=1) as consts,
            tc.tile_pool(name="xbf", bufs=6) as xbf_pool,
            tc.tile_pool(name="dwf", bufs=3) as dwf_pool,
            tc.tile_pool(name="lr", bufs=5) as lr_pool,
            tc.tile_pool(name="r9", bufs=4) as r9_pool,
            tc.tile_pool(name="outc", bufs=4) as out_pool,
            tc.tile_pool(name="ps1", bufs=2, space="PSUM") as ps1_pool,
            tc.tile_pool(name="ps2", bufs=2, space="PSUM") as ps2_pool,
        ):
            # first two chunk loads issued before the const DMAs so the
            # input stream starts immediately
            pre = []
            for i in range(2):
                p, c, lo, csz, _ = chunks[i]
                xbf = xbf_pool.tile([P, PCOLS], BF16, tag="xbf")
                nc.sync.dma_start(xbf[:, 0:PACKED[csz]],
                                  x0p[p, c, :, 0:PACKED[csz]])
                pre.append(xbf)

            diag_t = consts.tile([P, PAIRS * K * P], BF16)
            nc.scalar.dma_start(diag_t[:], diags[:])
            wblk_t = consts.tile([P, P], BF16)
            nc.scalar.dma_start(wblk_t[:], wblk[:])
            scal_t = consts.tile([P, 3 * PAIRS], F32)
            nc.scalar.dma_start(scal_t[:], scal[:])
            att = [scal_t[:, p:p + 1] for p in range(PAIRS)]
            pb = [scal_t[:, PAIRS + p:PAIRS + p + 1] for p in range(PAIRS)]
            k0 = [scal_t[:, 2 * PAIRS + p:2 * PAIRS + p + 1]
                  for p in range(PAIRS)]

            def lrelu(dst, src, bias):
                if use_lrelu:
                    nc.scalar.activation(dst, src, Prelu, bias=bias,
                                         alpha=0.1)
                else:
                    sz = dst.shape[-1]
                    tt = r9_pool.tile([P, ST], F32, tag="tt")
                    nc.scalar.activation(tt[:, :sz], src, Ident, bias=bias)
                    r9 = r9_pool.tile([P, ST], F32, tag="r9")
                    nc.scalar.activation(r9[:, :sz], tt[:, :sz], Relu,
                                         scale=0.9)
                    nc.vector.scalar_tensor_tensor(
                        dst, tt[:, :sz], 0.1, r9[:, :sz], op0=mult, op1=add)

            def stage1(i):
                p, c, lo, csz, tmode = chunks[i]
                if i < 2:
                    xbf = pre[i]
                else:
                    xbf = xbf_pool.tile([P, PCOLS], BF16, tag="xbf")
                    nc.sync.dma_start(xbf[:, 0:PACKED[csz]],
                                      x0p[p, c, :, 0:PACKED[csz]])

                lrs = []
                for off, sz in _subtiles(csz):
                    ps1 = ps1_pool.tile([P, ST], F32)
                    jl = (0, 1, 2) if tmode == "P" else (1, 2)
                    for j in jl:
                        for hlo in range(0, sz, NT):
                            hsz = min(NT, sz - hlo)
                            nc.tensor.matmul(
                                ps1[:, hlo:hlo + hsz],
                                diag_t[:, (p * K + j) * P:
                                       (p * K + j + 1) * P],
                                xbf[:, off + hlo + 1 + j:
                                    off + hlo + 1 + j + hsz],
                                start=(j == jl[0]), stop=(j == 2),
                            )
                    lr = lr_pool.tile([P, ST], BF16)
                    if tmode == "V":
                        dwf = dwf_pool.tile([P, ST], BF16)
                        nc.vector.scalar_tensor_tensor(
                            dwf[:, :sz], xbf[:, off + 1:off + 1 + sz],
                            k0[p], ps1[:, :sz], op0=mult, op1=add)
                        lrelu(lr[:, :sz], dwf[:, :sz], pb[p])
                    else:
                        lrelu(lr[:, :sz], ps1[:, :sz], pb[p])
                    lrs.append(lr)
                return xbf, lrs

            def stage2(i, xbf, lrs):
                p, c, lo, csz, _ = chunks[i]
                outc = out_pool.tile([P, CHUNK], BF16, tag="outc")
                for (off, sz), lr in zip(_subtiles(csz), lrs):
                    ps2 = ps2_pool.tile([P, ST], F32)
                    for hlo in range(0, sz, NT):
                        hsz = min(NT, sz - hlo)
                        nc.tensor.matmul(
                            ps2[:, hlo:hlo + hsz], wblk_t[:],
                            lr[:, hlo:hlo + hsz], start=True, stop=True)
                    nc.vector.scalar_tensor_tensor(
                        outc[:, off:off + sz],
                        xbf[:, off + 2:off + 2 + sz],
                        att[p], ps2[:, :sz], op0=mult, op1=add)
                nc.gpsimd.dma_start(out[p, :, lo:lo + csz], outc[:, :csz])

            prev = None
            for i in range(len(chunks)):
                cur = stage1(i)
                if prev is not None:
                    stage2(i - 1, *prev)
                prev = cur
            stage2(len(chunks) - 1, *prev)

    nc.compile()
    return nc


def _lrelu(x):
    return np.where(x >= 0, x, np.float32(0.1) * x)


def kernel(x0, x1, W1, W2, conv_w, conv_b, ca_w1, ca_w2):
    global LAST_RESULT
    x0 = np.ascontiguousarray(np.asarray(x0, dtype=np.float32))
    x1 = np.asarray(x1, dtype=np.float32)
    W1 = np.asarray(W1, dtype=np.float32)
    W2 = np.asarray(W2, dtype=np.float32)
    conv_w = np.asarray(conv_w, dtype=np.float32)
    conv_b = np.asarray(conv_b, dtype=np.float32)
    ca_w1 = np.asarray(ca_w1, dtype=np.float32)
    ca_w2 = np.asarray(ca_w2, dtype=np.float32)

    # dynamic depthwise kernels + SE gate (tiny, fp32 host math)
    h = _lrelu(x1 @ W1.T)                                   # [B, 64]
    kern = (h @ W2.T).reshape(B, C, K)                      # [B, C, K]
    att = 1.0 / (1.0 + np.exp(-(_lrelu(x1 @ ca_w1.T) @ ca_w2.T)))
    att = att.astype(np.float32)                            # [B, C]

    # block-diagonal 1x1-conv weight as lhsT: lhsT[k, m] = W[m, k]
    wblk_np = np.zeros((P, P), np.float32)
    wblk_np[:C, :C] = conv_w.T
    wblk_np[C:, C:] = conv_w.T
    wblk_np = wblk_np.astype(BF16_NP)

    key = (USE_LRELU, TAP_PATTERN)
    if key not in _COMPILED:
        _COMPILED[key] = _build_program(*key)
    nc = _COMPILED[key]

    biasP = np.tile(conv_b, 2).astype(np.float32)            # [P]
    los = np.cumsum([0] + CHUNK_SIZES[:-1])
    in_maps = []
    for core in range(N_CORES):
        s0 = core * SAMPLES_PER_CORE
        diags_np = np.zeros((P, PAIRS * K * P), np.float32)
        scal_np = np.empty((P, 3 * PAIRS), np.float32)
        dvals = np.empty((PAIRS, P), np.float32)
        for p in range(PAIRS):
            ka = kern[s0 + 2 * p]          # [C, K]
            kb = kern[s0 + 2 * p + 1]
            kern_bf = np.empty((P, K), np.float32)
            for j in range(K):
                s = (p * K + j) * P
                d = np.concatenate([ka[:, j], kb[:, j]])
                np.fill_diagonal(diags_np[:, s:s + P], d)
                kern_bf[:, j] = d.astype(BF16_NP).astype(np.float32)
            attp = np.concatenate([att[s0 + 2 * p], att[s0 + 2 * p + 1]])
            dp = biasP / attp                                 # [P]
            dvals[p] = dp
            scal_np[:, p] = attp
            # depthwise compensation: -sum_j bf16(kern_j) * d
            scal_np[:, PAIRS + p] = -(kern_bf.sum(axis=1) * dp)
            scal_np[:, 2 * PAIRS + p] = kern_bf[:, 0]
        x0c = x0[s0:s0 + SAMPLES_PER_CORE].reshape(PAIRS, P, L)
        x0c = (x0c + dvals[:, :, None]).astype(BF16_NP)
        # per-chunk packed rows: x0p[p, c, :, i] = x0d[lo - 2 + i], with the
        # out-of-range edge columns set to d (so padded taps cancel via pb)
        x0p_np = np.zeros((PAIRS, NCH, P, PCOLS), BF16_NP)
        dbf = dvals.astype(BF16_NP)                           # [PAIRS, P]
        for c, csz in enumerate(CHUNK_SIZES):
            lo = int(los[c])
            a, b = lo - 2, lo + csz + 2
            sa, sb = max(a, 0), min(b, L)
            x0p_np[:, c, :, sa - a:sb - a] = x0c[:, :, sa:sb]
            for i in range(a, sa):
                x0p_np[:, c, :, i - a] = dbf
            for i in range(sb, b):
                x0p_np[:, c, :, i - a] = dbf
        in_maps.append({
            "x0p": x0p_np,
            "diags": diags_np.astype(BF16_NP),
            "scal": scal_np,
            "wblk": wblk_np,
        })

    res = run_bass_kernel_spmd(nc, in_maps, list(range(N_CORES)), trace=TRACE)
    LAST_RESULT = res

    full = np.empty((B, C, L), np.float32)
    for core in range(N_CORES):
        s0 = core * SAMPLES_PER_CORE
        full[s0:s0 + SAMPLES_PER_CORE] = (
            res.results[core]["out"].astype(np.float32)
            .reshape(SAMPLES_PER_CORE, C, L))
    return full


# revision 16
# speedup vs baseline: 1.1187x; 1.0263x over previous
"""Trainium2 Bass kernel for nn_DA_conv1D (dynamic depthwise conv1d + 1x1 conv
+ channel-attention gate), data-parallel over batch on 8 NeuronCores.

Shapes (hardcoded): x0 [32, 64, 16384] f32, x1 [32, 64] f32.
Each core handles 4 samples, organized as 2 "pairs" of 2 samples so the
128 SBUF partitions hold (2 samples x 64 channels).

v3 design:
  * output stored in bf16, widened to f32 on host (halves store traffic).
  * DMA bodies are exactly 4096B per partition row (2048 cols bf16) and
    aligned — the 4100B packets of the +halo scheme ran ~20% slower.
    The 2 halo columns arrive via separate tiny DMAs on the gpsimd queue.
  * the two sample-pairs are interleaved chunk-by-chunk: two independent
    streams fill each other's pipeline bubbles (fill/drain and PSUM-dep
    stalls).
  * per-chunk engine balancing:
      tap mode   P: all 3 depthwise taps as PE diag-matmuls
                 V: taps 1,2 on PE; tap 0 folded into a DVE STT that
                    also evacuates ps1 to SBUF (Prelu then reads SBUF)
      evac mode  D: DVE STT reads conv PSUM directly (1x mode)
                 E: ACT copies conv PSUM to SBUF bf16, DVE STT then runs
                    all-SBUF at 2x
    Patterns chosen so PE / ACT / DVE land at roughly equal busy time.

Per 1024-wide subtile (SBUF layout [128 part, L free]):
  ps1 = sum_j diag(kern_j) @ x_shift_j   (PE, accumulating bf16 matmuls)
  [V]  dwf = k0*x_{-1} + ps1             (DVE STT, PSUM->SBUF bf16)
  lr  = Prelu(ps1|dwf + pb)              (ACT, bf16 out)
  ps2 = blockdiag(conv_w) @ lr           (PE bf16 matmuls)
  [E]  es  = copy(ps2)                   (ACT, PSUM->SBUF bf16)
  out = x0b * att + (ps2|es)             (DVE STT -> bf16 SBUF)

x0 is pre-shifted by d = conv_b/att and cast to bf16 on host; the gate
multiply then yields att*x0 + conv_b for free, and the Prelu bias
pb = -sum_j kern_j*d compensates the shift on the depthwise path.
"""

import os
import sys

for _p in ("/opt/trn_rl_repo", "/root/.axon_site/_ro/trn_rl_repo"):
    if os.path.isdir(_p) and _p not in sys.path:
        sys.path.append(_p)

import ml_dtypes
import numpy as np

import concourse.bacc as bacc
import concourse.tile as tile
from concourse import mybir
from concourse.bass_utils import run_bass_kernel_spmd

B, C, L, K = 32, 64, 16384, 3
N_CORES = 8
SAMPLES_PER_CORE = B // N_CORES          # 4
PAIRS = SAMPLES_PER_CORE // 2            # 2
P = 128                                  # SBUF partitions = 2 samples x 64 ch
CHUNK = 4096                             # max chunk (SBUF tile size)
# tapered; the input rows for each chunk are host-packed with their 2+2
# halo columns and padded to a 32-col (64B) multiple so every DMA packet
# is a 64B multiple (4100B packets measured ~20% slower than 4096B)
CHUNK_SIZES = [1024, 2048, 4096, 4096, 4096, 1024]
PACKED = {c: (c + 4 + 31) // 32 * 32 for c in CHUNK_SIZES}
PCOLS = max(PACKED.values())             # 4128 (8256B rows)
ST = 1024                                # ACT/DVE subtile width (2 PSUM banks)
NT = 512                                 # matmul moving width (PSUM bank)
F32 = mybir.dt.float32
BF16 = mybir.dt.bfloat16
BF16_NP = ml_dtypes.bfloat16

NCH = len(CHUNK_SIZES)                   # 6 chunks per pair
# per-chunk tap0 placement: P = tap0 on the PE, V = folded into DVE STT
# (global order interleaves the two pairs chunk-by-chunk)
TAP_PATTERN = "PPVVPPVVPPPP"

TRACE = False          # test harness flips this to profile
USE_LRELU = True       # HW Prelu activation (CoreSim lacks it; see simcheck)
LAST_RESULT = None     # BassKernelResults of the most recent run

_COMPILED = {}


def _subtiles(csz):
    return [(o, min(ST, csz - o)) for o in range(0, csz, ST)]


def _build_program(use_lrelu, tap_pattern):
    nc = bacc.Bacc("TRN2", target_bir_lowering=False, debug=False,
                   num_devices=N_CORES)

    # host-packed input: chunk c's rows hold x0d[lo-2 : lo+csz+2] (+pad),
    # so xbf[:, i] = x0d[lo + i - 2]
    x0p = nc.dram_tensor("x0p", [PAIRS, NCH, P, PCOLS], BF16,
                         kind="ExternalInput").ap()
    # diag kernels pre-flattened per partition: [(pair, tap) -> 128 cols]
    diags = nc.dram_tensor("diags", [P, PAIRS * K * P], BF16,
                           kind="ExternalInput").ap()
    # scal columns: [att(pair), pb(pair), k0(pair)] where pb is the prelu
    # bias compensating the host-side x0 + d shift, k0 the tap-0 kernel
    scal = nc.dram_tensor("scal", [P, 3 * PAIRS], F32,
                          kind="ExternalInput").ap()
    wblk = nc.dram_tensor("wblk", [P, P], BF16, kind="ExternalInput").ap()
    out = nc.dram_tensor("out", [PAIRS, P, L], BF16, kind="ExternalOutput").ap()

    mult = mybir.AluOpType.mult
    add = mybir.AluOpType.add
    Relu = mybir.ActivationFunctionType.Relu
    Prelu = mybir.ActivationFunctionType.Prelu
    Ident = mybir.ActivationFunctionType.Identity

    # interleave the two pairs: (pair, chunk_idx, lo, csz, tap_mode)
    chunks = []
    for c, csz in enumerate(CHUNK_SIZES):
        lo = sum(CHUNK_SIZES[:c])
        for p in range(PAIRS):
            chunks.append((p, c, lo, csz))
    chunks = [ch + (tap_pattern[i],) for i, ch in enumerate(chunks)]

    with tile.TileContext(nc) as tc:
        with (
            tc.tile_pool(name="consts", bufs=1) as consts,
            tc.tile_pool(name="xbf", bufs=6) as xbf_pool,
            tc.tile_pool(name="dwf", bufs=3) as dwf_pool,
            tc.tile_pool(name="lr", bufs=5) as lr_pool,
            tc.tile_pool(name="r9", bufs=4) as r9_pool,
            tc.tile_pool(name="outc", bufs=4) as out_pool,
            tc.tile_pool(name="ps1", bufs=2, space="PSUM") as ps1_pool,
            tc.tile_pool(name="ps2", bufs=2, space="PSUM") as ps2_pool,
        ):
            # first two chunk loads issued before the const DMAs so the
            # input stream starts immediately
            pre = []
            for i in range(2):
                p, c, lo, csz, _ = chunks[i]
                xbf = xbf_pool.tile([P, PCOLS], BF16, tag="xbf")
                nc.sync.dma_start(xbf[:, 0:PACKED[csz]],
                                  x0p[p, c, :, 0:PACKED[csz]])
                pre.append(xbf)

            # scal first: the first Prelu's bias dep is on the critical path
            scal_t = consts.tile([P, 3 * PAIRS], F32)
            nc.scalar.dma_start(scal_t[:], scal[:])
            diag_t = consts.tile([P, PAIRS * K * P], BF16)
            nc.scalar.dma_start(diag_t[:], diags[:])
            wblk_t = consts.tile([P, P], BF16)
            nc.scalar.dma_start(wblk_t[:], wblk[:])
            # warm the ACT function-table (≈2.7us load) off the critical path
            warm = consts.tile([P, 1], F32)
            nc.scalar.activation(warm[:], scal_t[:, 0:1],
                                 mybir.ActivationFunctionType.Prelu,
                                 alpha=0.1)
            att = [scal_t[:, p:p + 1] for p in range(PAIRS)]
            pb = [scal_t[:, PAIRS + p:PAIRS + p + 1] for p in range(PAIRS)]
            k0 = [scal_t[:, 2 * PAIRS + p:2 * PAIRS + p + 1]
                  for p in range(PAIRS)]

            def lrelu(dst, src, bias):
                if use_lrelu:
                    nc.scalar.activation(dst, src, Prelu, bias=bias,
                                         alpha=0.1)
                else:
                    sz = dst.shape[-1]
                    tt = r9_pool.tile([P, ST], F32, tag="tt")
                    nc.scalar.activation(tt[:, :sz], src, Ident, bias=bias)
                    r9 = r9_pool.tile([P, ST], F32, tag="r9")
                    nc.scalar.activation(r9[:, :sz], tt[:, :sz], Relu,
                                         scale=0.9)
                    nc.vector.scalar_tensor_tensor(
                        dst, tt[:, :sz], 0.1, r9[:, :sz], op0=mult, op1=add)

            def stage1(i):
                p, c, lo, csz, tmode = chunks[i]
                if i < 2:
                    xbf = pre[i]
                else:
                    xbf = xbf_pool.tile([P, PCOLS], BF16, tag="xbf")
                    nc.sync.dma_start(xbf[:, 0:PACKED[csz]],
                                      x0p[p, c, :, 0:PACKED[csz]])

                lrs = []
                for off, sz in _subtiles(csz):
                    ps1 = ps1_pool.tile([P, ST], F32)
                    jl = (0, 1, 2) if tmode == "P" else (1, 2)
                    for j in jl:
                        for hlo in range(0, sz, NT):
                            hsz = min(NT, sz - hlo)
                            nc.tensor.matmul(
                                ps1[:, hlo:hlo + hsz],
                                diag_t[:, (p * K + j) * P:
                                       (p * K + j + 1) * P],
                                xbf[:, off + hlo + 1 + j:
                                    off + hlo + 1 + j + hsz],
                                start=(j == jl[0]), stop=(j == 2),
                            )
                    lr = lr_pool.tile([P, ST], BF16)
                    if tmode == "V":
                        dwf = dwf_pool.tile([P, ST], BF16)
                        nc.vector.scalar_tensor_tensor(
                            dwf[:, :sz], xbf[:, off + 1:off + 1 + sz],
                            k0[p], ps1[:, :sz], op0=mult, op1=add)
                        lrelu(lr[:, :sz], dwf[:, :sz], pb[p])
                    else:
                        lrelu(lr[:, :sz], ps1[:, :sz], pb[p])
                    lrs.append(lr)
                return xbf, lrs

            def stage2(i, xbf, lrs):
                p, c, lo, csz, _ = chunks[i]
                outc = out_pool.tile([P, CHUNK], BF16, tag="outc")
                for (off, sz), lr in zip(_subtiles(csz), lrs):
                    ps2 = ps2_pool.tile([P, ST], F32)
                    for hlo in range(0, sz, NT):
                        hsz = min(NT, sz - hlo)
                        nc.tensor.matmul(
                            ps2[:, hlo:hlo + hsz], wblk_t[:],
                            lr[:, hlo:hlo + hsz], start=True, stop=True)
                    nc.vector.scalar_tensor_tensor(
                        outc[:, off:off + sz],
                        xbf[:, off + 2:off + 2 + sz],
                        att[p], ps2[:, :sz], op0=mult, op1=add)
                nc.gpsimd.dma_start(out[p, :, lo:lo + csz], outc[:, :csz])

            prev = None
            for i in range(len(chunks)):
                cur = stage1(i)
                if prev is not None:
                    stage2(i - 1, *prev)
                prev = cur
            stage2(len(chunks) - 1, *prev)

    nc.compile()
    return nc


def _lrelu(x):
    return np.where(x >= 0, x, np.float32(0.1) * x)


def kernel(x0, x1, W1, W2, conv_w, conv_b, ca_w1, ca_w2):
    global LAST_RESULT
    x0 = np.ascontiguousarray(np.asarray(x0, dtype=np.float32))
    x1 = np.asarray(x1, dtype=np.float32)
    W1 = np.asarray(W1, dtype=np.float32)
    W2 = np.asarray(W2, dtype=np.float32)
    conv_w = np.asarray(conv_w, dtype=np.float32)
    conv_b = np.asarray(conv_b, dtype=np.float32)
    ca_w1 = np.asarray(ca_w1, dtype=np.float32)
    ca_w2 = np.asarray(ca_w2, dtype=np.float32)

    # dynamic depthwise kernels + SE gate (tiny, fp32 host math)
    h = _lrelu(x1 @ W1.T)                                   # [B, 64]
    kern = (h @ W2.T).reshape(B, C, K)                      # [B, C, K]
    att = 1.0 / (1.0 + np.exp(-(_lrelu(x1 @ ca_w1.T) @ ca_w2.T)))
    att = att.astype(np.float32)                            # [B, C]

    # block-diagonal 1x1-conv weight as lhsT: lhsT[k, m] = W[m, k]
    wblk_np = np.zeros((P, P), np.float32)
    wblk_np[:C, :C] = conv_w.T
    wblk_np[C:, C:] = conv_w.T
    wblk_np = wblk_np.astype(BF16_NP)

    key = (USE_LRELU, TAP_PATTERN)
    if key not in _COMPILED:
        _COMPILED[key] = _build_program(*key)
    nc = _COMPILED[key]

    biasP = np.tile(conv_b, 2).astype(np.float32)            # [P]
    los = np.cumsum([0] + CHUNK_SIZES[:-1])
    in_maps = []
    for core in range(N_CORES):
        s0 = core * SAMPLES_PER_CORE
        diags_np = np.zeros((P, PAIRS * K * P), np.float32)
        scal_np = np.empty((P, 3 * PAIRS), np.float32)
        dvals = np.empty((PAIRS, P), np.float32)
        for p in range(PAIRS):
            ka = kern[s0 + 2 * p]          # [C, K]
            kb = kern[s0 + 2 * p + 1]
            kern_bf = np.empty((P, K), np.float32)
            for j in range(K):
                s = (p * K + j) * P
                d = np.concatenate([ka[:, j], kb[:, j]])
                np.fill_diagonal(diags_np[:, s:s + P], d)
                kern_bf[:, j] = d.astype(BF16_NP).astype(np.float32)
            attp = np.concatenate([att[s0 + 2 * p], att[s0 + 2 * p + 1]])
            dp = biasP / attp                                 # [P]
            dvals[p] = dp
            scal_np[:, p] = attp
            # depthwise compensation: -sum_j bf16(kern_j) * d
            scal_np[:, PAIRS + p] = -(kern_bf.sum(axis=1) * dp)
            scal_np[:, 2 * PAIRS + p] = kern_bf[:, 0]
        x0c = x0[s0:s0 + SAMPLES_PER_CORE].reshape(PAIRS, P, L)
        x0c = (x0c + dvals[:, :, None]).astype(BF16_NP)
        # per-chunk packed rows: x0p[p, c, :, i] = x0d[lo - 2 + i], with the
        # out-of-range edge columns set to d (so padded taps cancel via pb)
        x0p_np = np.zeros((PAIRS, NCH, P, PCOLS), BF16_NP)
        dbf = dvals.astype(BF16_NP)                           # [PAIRS, P]
        for c, csz in enumerate(CHUNK_SIZES):
            lo = int(los[c])
            a, b = lo - 2, lo + csz + 2
            sa, sb = max(a, 0), min(b, L)
            x0p_np[:, c, :, sa - a:sb - a] = x0c[:, :, sa:sb]
            for i in range(a, sa):
                x0p_np[:, c, :, i - a] = dbf
            for i in range(sb, b):
                x0p_np[:, c, :, i - a] = dbf
        in_maps.append({
            "x0p": x0p_np,
            "diags": diags_np.astype(BF16_NP),
            "scal": scal_np,
            "wblk": wblk_np,
        })

    res = run_bass_kernel_spmd(nc, in_maps, list(range(N_CORES)), trace=TRACE)
    LAST_RESULT = res

    full = np.empty((B, C, L), np.float32)
    for core in range(N_CORES):
        s0 = core * SAMPLES_PER_CORE
        full[s0:s0 + SAMPLES_PER_CORE] = (
            res.results[core]["out"].astype(np.float32)
            .reshape(SAMPLES_PER_CORE, C, L))
    return full


# revision 20
# speedup vs baseline: 1.1649x; 1.0413x over previous
"""Trainium2 Bass kernel for nn_DA_conv1D (dynamic depthwise conv1d + 1x1 conv
+ channel-attention gate), data-parallel over batch on 8 NeuronCores.

Shapes (hardcoded): x0 [32, 64, 16384] f32, x1 [32, 64] f32.
Each core handles 4 samples, organized as 2 "pairs" of 2 samples so the
128 SBUF partitions hold (2 samples x 64 channels).

v3 design:
  * output stored in bf16, widened to f32 on host (halves store traffic).
  * DMA bodies are exactly 4096B per partition row (2048 cols bf16) and
    aligned — the 4100B packets of the +halo scheme ran ~20% slower.
    The 2 halo columns arrive via separate tiny DMAs on the gpsimd queue.
  * the two sample-pairs are interleaved chunk-by-chunk: two independent
    streams fill each other's pipeline bubbles (fill/drain and PSUM-dep
    stalls).
  * per-chunk engine balancing:
      tap mode   P: all 3 depthwise taps as PE diag-matmuls
                 V: taps 1,2 on PE; tap 0 folded into a DVE STT that
                    also evacuates ps1 to SBUF (Prelu then reads SBUF)
      evac mode  D: DVE STT reads conv PSUM directly (1x mode)
                 E: ACT copies conv PSUM to SBUF bf16, DVE STT then runs
                    all-SBUF at 2x
    Patterns chosen so PE / ACT / DVE land at roughly equal busy time.

Per 1024-wide subtile (SBUF layout [128 part, L free]):
  ps1 = sum_j diag(kern_j) @ x_shift_j   (PE, accumulating bf16 matmuls)
  [V]  dwf = k0*x_{-1} + ps1             (DVE STT, PSUM->SBUF bf16)
  lr  = Prelu(ps1|dwf + pb)              (ACT, bf16 out)
  ps2 = blockdiag(conv_w) @ lr           (PE bf16 matmuls)
  [E]  es  = copy(ps2)                   (ACT, PSUM->SBUF bf16)
  out = x0b * att + (ps2|es)             (DVE STT -> bf16 SBUF)

x0 is pre-shifted by d = conv_b/att and cast to bf16 on host; the gate
multiply then yields att*x0 + conv_b for free, and the Prelu bias
pb = -sum_j kern_j*d compensates the shift on the depthwise path.
"""

import os
import sys

for _p in ("/opt/trn_rl_repo", "/root/.axon_site/_ro/trn_rl_repo"):
    if os.path.isdir(_p) and _p not in sys.path:
        sys.path.append(_p)

import ml_dtypes
import numpy as np

import concourse.bacc as bacc
import concourse.tile as tile
from concourse import mybir
from concourse.bass_utils import run_bass_kernel_spmd

B, C, L, K = 32, 64, 16384, 3
N_CORES = 8
SAMPLES_PER_CORE = B // N_CORES          # 4
PAIRS = SAMPLES_PER_CORE // 2            # 2
P = 128                                  # SBUF partitions = 2 samples x 64 ch
CHUNK = 4096                             # max chunk (SBUF tile size)
# tapered; the input rows for each chunk are host-packed with their 2+2
# halo columns and padded to a 32-col (64B) multiple so every DMA packet
# is a 64B multiple (4100B packets measured ~20% slower than 4096B)
CHUNK_SIZES = [1024, 2048, 4096, 4096, 4096, 1024]
PACKED = {c: (c + 4 + 31) // 32 * 32 for c in CHUNK_SIZES}
PCOLS = max(PACKED.values())             # 4128 (8256B rows)
ST = 1024                                # ACT/DVE subtile width (2 PSUM banks)
NT = 512                                 # matmul moving width (PSUM bank)
F32 = mybir.dt.float32
BF16 = mybir.dt.bfloat16
BF16_NP = ml_dtypes.bfloat16

NCH = len(CHUNK_SIZES)                   # 6 chunks per pair
# per-SUBTILE tap0 placement, cycled globally: P = tap0 on the PE (3rd diag
# matmul), V = folded into the DVE STT. Fine-grained interleave keeps PE and
# DVE simultaneously busy (a per-chunk pattern made them alternate idling).
# Balance: P-subtile PE≈1.75us/DVE≈1.28; V-subtile PE≈1.31/DVE≈2.55
# -> equal engine time at ~72% P.
TAP_PATTERN = "PPVPPVP"

TRACE = False          # test harness flips this to profile
USE_LRELU = True       # HW Prelu activation (CoreSim lacks it; see simcheck)
LAST_RESULT = None     # BassKernelResults of the most recent run

_COMPILED = {}


def _subtiles(csz):
    return [(o, min(ST, csz - o)) for o in range(0, csz, ST)]


def _build_program(use_lrelu, tap_pattern):
    nc = bacc.Bacc("TRN2", target_bir_lowering=False, debug=False,
                   num_devices=N_CORES)

    # host-packed input: chunk c's rows hold x0d[lo-2 : lo+csz+2] (+pad),
    # so xbf[:, i] = x0d[lo + i - 2]
    x0p = nc.dram_tensor("x0p", [PAIRS, NCH, P, PCOLS], BF16,
                         kind="ExternalInput").ap()
    # diag kernels pre-flattened per partition: [(pair, tap) -> 128 cols]
    diags = nc.dram_tensor("diags", [P, PAIRS * K * P], BF16,
                           kind="ExternalInput").ap()
    # scal columns: [att(pair), pb(pair), k0(pair)] where pb is the prelu
    # bias compensating the host-side x0 + d shift, k0 the tap-0 kernel
    scal = nc.dram_tensor("scal", [P, 3 * PAIRS], F32,
                          kind="ExternalInput").ap()
    wblk = nc.dram_tensor("wblk", [P, P], BF16, kind="ExternalInput").ap()
    out = nc.dram_tensor("out", [PAIRS, P, L], BF16, kind="ExternalOutput").ap()

    mult = mybir.AluOpType.mult
    add = mybir.AluOpType.add
    Relu = mybir.ActivationFunctionType.Relu
    Prelu = mybir.ActivationFunctionType.Prelu
    Ident = mybir.ActivationFunctionType.Identity

    # interleave the two pairs: (pair, chunk_idx, lo, csz)
    chunks = []
    for c, csz in enumerate(CHUNK_SIZES):
        lo = sum(CHUNK_SIZES[:c])
        for p in range(PAIRS):
            chunks.append((p, c, lo, csz))
    sub_n = [0]                      # global subtile counter for mode cycling
    n_subs = sum(csz // ST for _, _, _, csz in chunks)

    def tap_mode():
        i = sub_n[0]
        sub_n[0] += 1
        if i >= n_subs - 2:
            return "P"               # keep the drain tail off the DVE
        return tap_pattern[i % len(tap_pattern)]

    with tile.TileContext(nc) as tc:
        with (
            tc.tile_pool(name="consts", bufs=1) as consts,
            tc.tile_pool(name="xbf", bufs=8) as xbf_pool,
            tc.tile_pool(name="dwf", bufs=3) as dwf_pool,
            tc.tile_pool(name="lr", bufs=5) as lr_pool,
            tc.tile_pool(name="r9", bufs=4) as r9_pool,
            tc.tile_pool(name="outc", bufs=4) as out_pool,
            tc.tile_pool(name="ps1", bufs=2, space="PSUM") as ps1_pool,
            tc.tile_pool(name="ps2", bufs=2, space="PSUM") as ps2_pool,
        ):
            # first two chunk loads issued before the const DMAs so the
            # input stream starts immediately
            pre = []
            for i in range(2):
                p, c, lo, csz = chunks[i]
                xbf = xbf_pool.tile([P, PCOLS], BF16, tag="xbf")
                nc.sync.dma_start(xbf[:, 0:PACKED[csz]],
                                  x0p[p, c, :, 0:PACKED[csz]])
                pre.append(xbf)

            # scal first: the first Prelu's bias dep is on the critical path
            scal_t = consts.tile([P, 3 * PAIRS], F32)
            nc.scalar.dma_start(scal_t[:], scal[:])
            diag_t = consts.tile([P, PAIRS * K * P], BF16)
            nc.scalar.dma_start(diag_t[:], diags[:])
            wblk_t = consts.tile([P, P], BF16)
            nc.scalar.dma_start(wblk_t[:], wblk[:])
            # warm the ACT function-table (≈2.7us load) off the critical path
            warm = consts.tile([P, 1], F32)
            nc.scalar.activation(warm[:], scal_t[:, 0:1],
                                 mybir.ActivationFunctionType.Prelu,
                                 alpha=0.1)
            att = [scal_t[:, p:p + 1] for p in range(PAIRS)]
            pb = [scal_t[:, PAIRS + p:PAIRS + p + 1] for p in range(PAIRS)]
            k0 = [scal_t[:, 2 * PAIRS + p:2 * PAIRS + p + 1]
                  for p in range(PAIRS)]

            def lrelu(dst, src, bias):
                if use_lrelu:
                    nc.scalar.activation(dst, src, Prelu, bias=bias,
                                         alpha=0.1)
                else:
                    sz = dst.shape[-1]
                    tt = r9_pool.tile([P, ST], F32, tag="tt")
                    nc.scalar.activation(tt[:, :sz], src, Ident, bias=bias)
                    r9 = r9_pool.tile([P, ST], F32, tag="r9")
                    nc.scalar.activation(r9[:, :sz], tt[:, :sz], Relu,
                                         scale=0.9)
                    nc.vector.scalar_tensor_tensor(
                        dst, tt[:, :sz], 0.1, r9[:, :sz], op0=mult, op1=add)

            def stage1(i):
                p, c, lo, csz = chunks[i]
                if i < 2:
                    xbf = pre[i]
                else:
                    xbf = xbf_pool.tile([P, PCOLS], BF16, tag="xbf")
                    nc.sync.dma_start(xbf[:, 0:PACKED[csz]],
                                      x0p[p, c, :, 0:PACKED[csz]])

                lrs = []
                for off, sz in _subtiles(csz):
                    tmode = tap_mode()
                    ps1 = ps1_pool.tile([P, ST], F32)
                    jl = (0, 1, 2) if tmode == "P" else (1, 2)
                    for j in jl:
                        for hlo in range(0, sz, NT):
                            hsz = min(NT, sz - hlo)
                            nc.tensor.matmul(
                                ps1[:, hlo:hlo + hsz],
                                diag_t[:, (p * K + j) * P:
                                       (p * K + j + 1) * P],
                                xbf[:, off + hlo + 1 + j:
                                    off + hlo + 1 + j + hsz],
                                start=(j == jl[0]), stop=(j == 2),
                            )
                    lr = lr_pool.tile([P, ST], BF16)
                    if tmode == "V":
                        dwf = dwf_pool.tile([P, ST], BF16)
                        nc.vector.scalar_tensor_tensor(
                            dwf[:, :sz], xbf[:, off + 1:off + 1 + sz],
                            k0[p], ps1[:, :sz], op0=mult, op1=add)
                        lrelu(lr[:, :sz], dwf[:, :sz], pb[p])
                    else:
                        lrelu(lr[:, :sz], ps1[:, :sz], pb[p])
                    lrs.append(lr)
                return xbf, lrs

            def stage2(i, xbf, lrs):
                p, c, lo, csz = chunks[i]
                outc = out_pool.tile([P, CHUNK], BF16, tag="outc")
                for (off, sz), lr in zip(_subtiles(csz), lrs):
                    ps2 = ps2_pool.tile([P, ST], F32)
                    for hlo in range(0, sz, NT):
                        hsz = min(NT, sz - hlo)
                        nc.tensor.matmul(
                            ps2[:, hlo:hlo + hsz], wblk_t[:],
                            lr[:, hlo:hlo + hsz], start=True, stop=True)
                    nc.vector.scalar_tensor_tensor(
                        outc[:, off:off + sz],
                        xbf[:, off + 2:off + 2 + sz],
                        att[p], ps2[:, :sz], op0=mult, op1=add)
                nc.gpsimd.dma_start(out[p, :, lo:lo + csz], outc[:, :csz])

            prev = None
            for i in range(len(chunks)):
                cur = stage1(i)
                if prev is not None:
                    stage2(i - 1, *prev)
                prev = cur
            stage2(len(chunks) - 1, *prev)

    nc.compile()
    return nc


def _lrelu(x):
    return np.where(x >= 0, x, np.float32(0.1) * x)


def kernel(x0, x1, W1, W2, conv_w, conv_b, ca_w1, ca_w2):
    global LAST_RESULT
    x0 = np.ascontiguousarray(np.asarray(x0, dtype=np.float32))
    x1 = np.asarray(x1, dtype=np.float32)
    W1 = np.asarray(W1, dtype=np.float32)
    W2 = np.asarray(W2, dtype=np.float32)
    conv_w = np.asarray(conv_w, dtype=np.float32)
    conv_b = np.asarray(conv_b, dtype=np.float32)
    ca_w1 = np.asarray(ca_w1, dtype=np.float32)
    ca_w2 = np.asarray(ca_w2, dtype=np.float32)

    # dynamic depthwise kernels + SE gate (tiny, fp32 host math)
    h = _lrelu(x1 @ W1.T)                                   # [B, 64]
    kern = (h @ W2.T).reshape(B, C, K)                      # [B, C, K]
    att = 1.0 / (1.0 + np.exp(-(_lrelu(x1 @ ca_w1.T) @ ca_w2.T)))
    att = att.astype(np.float32)                            # [B, C]

    # block-diagonal 1x1-conv weight as lhsT: lhsT[k, m] = W[m, k]
    wblk_np = np.zeros((P, P), np.float32)
    wblk_np[:C, :C] = conv_w.T
    wblk_np[C:, C:] = conv_w.T
    wblk_np = wblk_np.astype(BF16_NP)

    key = (USE_LRELU, TAP_PATTERN)
    if key not in _COMPILED:
        _COMPILED[key] = _build_program(*key)
    nc = _COMPILED[key]

    biasP = np.tile(conv_b, 2).astype(np.float32)            # [P]
    los = np.cumsum([0] + CHUNK_SIZES[:-1])
    in_maps = []
    for core in range(N_CORES):
        s0 = core * SAMPLES_PER_CORE
        diags_np = np.zeros((P, PAIRS * K * P), np.float32)
        scal_np = np.empty((P, 3 * PAIRS), np.float32)
        dvals = np.empty((PAIRS, P), np.float32)
        for p in range(PAIRS):
            ka = kern[s0 + 2 * p]          # [C, K]
            kb = kern[s0 + 2 * p + 1]
            kern_bf = np.empty((P, K), np.float32)
            for j in range(K):
                s = (p * K + j) * P
                d = np.concatenate([ka[:, j], kb[:, j]])
                np.fill_diagonal(diags_np[:, s:s + P], d)
                kern_bf[:, j] = d.astype(BF16_NP).astype(np.float32)
            attp = np.concatenate([att[s0 + 2 * p], att[s0 + 2 * p + 1]])
            dp = biasP / attp                                 # [P]
            dvals[p] = dp
            scal_np[:, p] = attp
            # depthwise compensation: -sum_j bf16(kern_j) * d
            scal_np[:, PAIRS + p] = -(kern_bf.sum(axis=1) * dp)
            scal_np[:, 2 * PAIRS + p] = kern_bf[:, 0]
        x0c = x0[s0:s0 + SAMPLES_PER_CORE].reshape(PAIRS, P, L)
        x0c = (x0c + dvals[:, :, None]).astype(BF16_NP)
        # per-chunk packed rows: x0p[p, c, :, i] = x0d[lo - 2 + i], with the
        # out-of-range edge columns set to d (so padded taps cancel via pb)
        x0p_np = np.zeros((PAIRS, NCH, P, PCOLS), BF16_NP)
        dbf = dvals.astype(BF16_NP)                           # [PAIRS, P]
        for c, csz in enumerate(CHUNK_SIZES):
            lo = int(los[c])
            a, b = lo - 2, lo + csz + 2
            sa, sb = max(a, 0), min(b, L)
            x0p_np[:, c, :, sa - a:sb - a] = x0c[:, :, sa:sb]
            for i in range(a, sa):
                x0p_np[:, c, :, i - a] = dbf
            for i in range(sb, b):
                x0p_np[:, c, :, i - a] = dbf
        in_maps.append({
            "x0p": x0p_np,
            "diags": diags_np.astype(BF16_NP),
            "scal": scal_np,
            "wblk": wblk_np,
        })

    res = run_bass_kernel_spmd(nc, in_maps, list(range(N_CORES)), trace=TRACE)
    LAST_RESULT = res

    full = np.empty((B, C, L), np.float32)
    for core in range(N_CORES):
        s0 = core * SAMPLES_PER_CORE
        full[s0:s0 + SAMPLES_PER_CORE] = (
            res.results[core]["out"].astype(np.float32)
            .reshape(SAMPLES_PER_CORE, C, L))
    return full
